# revision 1
# baseline (speedup 1.0000x reference)
"""Trainium2 Bass kernel for nn_AttentionPoolingTemporalEncoder.

Strategy (data-parallel over batch, 8 cores, 4 batch rows each):
  device:  h = relu(x @ Wp)               (bf16 matmuls, x pre-transposed on host)
           scores = h @ ((Wk @ qh)/sqrt(D))   (bk shifts cancel in softmax)
           p = exp(scores + maskbias)     (no running max; scores are O(5))
           U[h,:] = sum_s p[s,h] * h[s,:] ; Z[h] = sum_s p[s,h]
  host:    pooled = (U/Z) @ Wv (+bv) per head; @Wo+bo; @W2+b2; LayerNorm.
"""

import sys
import threading

import numpy as np

sys.path.insert(0, "/opt/trn_rl_repo")

from contextlib import ExitStack

import concourse.tile as tile
from concourse import bacc, mybir
from concourse.bass_utils import run_bass_kernel_spmd
from concourse.masks import make_identity


def _ensure_axon_ntff_hook_module():
    """Some images lack ``antenv.axon_hooks``; concourse imports it
    unconditionally when tracing is requested (e.g. via BASS_TRACE).
    Provide a minimal stand-in so that path degrades to no-trace
    instead of crashing."""
    try:
        from antenv import axon_hooks  # noqa: F401

        return
    except ImportError:
        pass
    import types

    mod = types.ModuleType("antenv.axon_hooks")
    mod._hook = None

    def set_axon_ntff_profile_hook(h):
        mod._hook = h

    def get_axon_ntff_profile_hook():
        return mod._hook

    mod.set_axon_ntff_profile_hook = set_axon_ntff_profile_hook
    mod.get_axon_ntff_profile_hook = get_axon_ntff_profile_hook
    sys.modules["antenv.axon_hooks"] = mod
    try:
        import antenv

        antenv.axon_hooks = mod
    except ImportError:
        pass


_ensure_axon_ntff_hook_module()

# Problem sizes (hardcoded per spec)
B, S, IN_DIM, E, H = 32, 4096, 1024, 512, 8
D = E // H
NCORES = 8
P = 128

_nc_cache = {}
_nc_lock = threading.Lock()


def build_nc(BL=B // NCORES, S_=S, I_=IN_DIM, has_bp=False, no_mask=False, trace_label=""):
    """Build + compile the per-core Bass program.

    BL: batch rows per core. S_: sequence length. I_: input dim.
    has_bp: emit the extra K=1 matmul adding the input-projection bias.
    """
    key = (BL, S_, I_, has_bp, no_mask)
    with _nc_lock:
        if key in _nc_cache:
            return _nc_cache[key]

    IC = I_ // P        # input-dim chunks
    EC = E // P         # embed-dim chunks
    S_TILES = S_ // P   # sequence tiles per batch row
    S_BLK = min(1024, S_)
    BLKS = S_ // S_BLK
    TPB = S_BLK // P    # s-tiles per DMA block

    f32 = mybir.dt.float32
    bf16 = mybir.dt.bfloat16
    RELU = mybir.ActivationFunctionType.Relu
    EXP = mybir.ActivationFunctionType.Exp
    COPY = mybir.ActivationFunctionType.Copy

    nc = bacc.Bacc(
        "TRN2",
        target_bir_lowering=False,
        debug=False,
        enable_asserts=False,
        num_devices=NCORES,
    )

    # DRAM I/O (per-core shapes). Matmul operands are bf16 (host-cast):
    # halves HBM traffic and runs the PE at full rate.
    xt = nc.dram_tensor("xt", [BL, IC, P, S_], bf16, kind="ExternalInput").ap()
    wp = nc.dram_tensor("wp", [IC, P, E], bf16, kind="ExternalInput").ap()
    wkq = nc.dram_tensor("wkq", [EC, P, H], bf16, kind="ExternalInput").ap()
    mb = nc.dram_tensor("mb", [BL, P, S_TILES], f32, kind="ExternalInput").ap()
    if has_bp:
        bp_d = nc.dram_tensor("bp", [1, E], bf16, kind="ExternalInput").ap()
    u_out = nc.dram_tensor("u_out", [BL, H, E], f32, kind="ExternalOutput").ap()
    z_out = nc.dram_tensor("z_out", [BL, H, 1], f32, kind="ExternalOutput").ap()

    with tile.TileContext(nc) as tc, ExitStack() as ctx:
        const = ctx.enter_context(tc.tile_pool(name="const", bufs=1))
        xp = ctx.enter_context(tc.tile_pool(name="xp", bufs=3))
        hp = ctx.enter_context(tc.tile_pool(name="hp", bufs=4))
        htp = ctx.enter_context(tc.tile_pool(name="htp", bufs=3))
        pp = ctx.enter_context(tc.tile_pool(name="pp", bufs=3))
        mbp = ctx.enter_context(tc.tile_pool(name="mbp", bufs=2))
        uzp = ctx.enter_context(tc.tile_pool(name="uzp", bufs=2))
        ps_h = ctx.enter_context(tc.tile_pool(name="ps_h", bufs=4, space="PSUM"))
        ps_s = ctx.enter_context(tc.tile_pool(name="ps_s", bufs=1, space="PSUM"))
        ps_u = ctx.enter_context(tc.tile_pool(name="ps_u", bufs=2, space="PSUM"))
        ps_z = ctx.enter_context(tc.tile_pool(name="ps_z", bufs=1, space="PSUM"))

        # Resident constants
        wp_sb = const.tile([P, IC, E], bf16)
        nc.sync.dma_start(wp_sb[:], wp.rearrange("c p e -> p c e"))
        wkq_sb = const.tile([P, EC, H], bf16)
        nc.sync.dma_start(wkq_sb[:], wkq.rearrange("c p h -> p c h"))
        ones_t = const.tile([P, 2], bf16)
        nc.gpsimd.memset(ones_t[:], 1.0)
        if has_bp:
            ones_row = const.tile([1, P], bf16)
            nc.gpsimd.memset(ones_row[:], 1.0)
            bp_sb = const.tile([1, E], bf16)
            nc.sync.dma_start(bp_sb[:], bp_d[:])

        # Chunked x prefetch (1024 s = 8 tiles per chunk), issued ahead so
        # loads never queue behind the per-tile transposes.
        SC = min(1024, S_)
        NCH = S_ // SC
        chunks = [(bb, cc) for bb in range(BL) for cc in range(NCH)]

        def load_chunk(idx):
            bb, cc = chunks[idx]
            xt_c = xp.tile([P, IC, SC], bf16, tag="xchunk")
            nc.sync.dma_start(
                xt_c[:],
                xt[bb, :, :, cc * SC : (cc + 1) * SC].rearrange("c p s -> p c s"),
            )
            return xt_c

        # distance-2 prefetch: two chunks in flight ahead of the consumer
        bufq = [load_chunk(0)]
        if len(chunks) > 1:
            bufq.append(load_chunk(1))
        chunk_idx = 1

        for b in range(BL):
            mb_t = mbp.tile([P, S_TILES], f32)
            nc.gpsimd.dma_start(mb_t[:], mb[b])
            u_ps = ps_u.tile([H, E], f32)
            z_ps = ps_z.tile([H, 2], f32)

            # Software-pipelined tails (depth 2): tile t's attention tail
            # (scores/exp/U/Z) is emitted 1-2 tiles later; all pending tails
            # flush right before a chunk boundary so the PE has work to chew
            # while a late x-chunk DMA completes.
            pending = []  # [(t, h_se, ht_sb), ...]

            def emit_tail(pend):
                t_, h_se_, ht_sb_ = pend
                # scores[s,h] = sum_e h[s,e] wkq[e,h]
                sc_ps = ps_s.tile([P, H], f32)
                for ec in range(EC):
                    nc.tensor.matmul(
                        sc_ps[:],
                        ht_sb_[:, ec, :],
                        wkq_sb[:, ec, :],
                        start=(ec == 0),
                        stop=(ec == EC - 1),
                    )
                # p = exp(scores + maskbias); maskbias = 0 for unmasked, -1e4
                # for masked positions (additive bias port, per-partition).
                p_sb = pp.tile([P, H], bf16)
                nc.scalar.activation(
                    p_sb[:], sc_ps[:], EXP, bias=mb_t[:, t_ : t_ + 1]
                )
                nc.tensor.matmul(
                    u_ps[:],
                    p_sb[:],
                    h_se_[:],
                    start=(t_ == 0),
                    stop=(t_ == S_TILES - 1),
                    skip_group_check=True,
                )
                nc.tensor.matmul(
                    z_ps[:],
                    p_sb[:],
                    ones_t[:],
                    start=(t_ == 0),
                    stop=(t_ == S_TILES - 1),
                    skip_group_check=True,
                )

            for t in range(S_TILES):
                    TPC = SC // P
                    if t % TPC == 0:
                        # consume the next chunk; keep two loads in flight
                        x_sb = bufq.pop(0)
                        if chunk_idx + 1 < len(chunks):
                            chunk_idx += 1
                            bufq.append(load_chunk(chunk_idx))
                    # h = relu(x @ Wp): accumulate 8 i-chunks into PSUM
                    h_ps = ps_h.tile([P, E], f32)
                    for c in range(IC):
                        nc.tensor.matmul(
                            h_ps[:],
                            x_sb[:, c, (t % TPC) * P : (t % TPC + 1) * P],
                            wp_sb[:, c, :],
                            start=(c == 0),
                            stop=(c == IC - 1) and not has_bp,
                        )
                    if has_bp:
                        nc.tensor.matmul(
                            h_ps[:],
                            ones_row[:],
                            bp_sb[:],
                            start=False,
                            stop=True,
                        )
                    h_se = hp.tile([P, E], bf16)
                    nc.scalar.activation(h_se[:], h_ps[:], RELU)

                    # hT via one batched DMA XBAR transpose, SBUF -> SBUF:
                    # ht_sb[e_in, ec, s] = h_se[s, ec*128 + e_in]
                    ht_sb = htp.tile([P, EC, P], bf16)
                    nc.sync.dma_start_transpose(ht_sb[:], h_se[:])

                    pending.append((t, h_se, ht_sb))
                    if len(pending) > 1:
                        emit_tail(pending.pop(0))
            while pending:
                emit_tail(pending.pop(0))

            u_sb = uzp.tile([H, E], f32, tag="u_sb")
            z_sb = uzp.tile([H, 1], f32, tag="z_sb")
            nc.vector.tensor_copy(u_sb[:], u_ps[:])
            nc.vector.tensor_copy(z_sb[:], z_ps[:, 0:1])
            nc.sync.dma_start(u_out[b], u_sb[:])
            nc.sync.dma_start(z_out[b], z_sb[:])

    nc.compile()
    with _nc_lock:
        _nc_cache[key] = nc
    return nc


def prepare_core_inputs(x, mask, Wp, wkq_scaled, bp=None):
    """Host-side packing for ONE core's shard.

    x: (BL, S, IN_DIM) fp32; mask: (BL, S) int; wkq_scaled: (E, H) fp32.
    """
    import ml_dtypes

    bf16 = ml_dtypes.bfloat16
    BL_, S_, I_ = x.shape
    IC = I_ // P
    EC = E // P
    # xt[b, c, i_in, s] = x[b, s, c*128+i_in]
    xt = np.ascontiguousarray(
        x.reshape(BL_, S_, IC, P).transpose(0, 2, 3, 1)
    ).astype(bf16)
    wp = np.ascontiguousarray(Wp.reshape(IC, P, E)).astype(bf16)
    wkq = np.ascontiguousarray(wkq_scaled.reshape(EC, P, H)).astype(bf16)
    # additive mask bias packed [BL, P, S_TILES]: 0 where kept, -1e4 where
    # masked (exp(-1e4 + s) underflows to exactly 0)
    mb = np.ascontiguousarray(
        ((mask.astype(np.float32) - 1.0) * 1.0e4)
        .reshape(BL_, S_ // P, P)
        .transpose(0, 2, 1)
    ).astype(np.float32)
    m = {"xt": xt, "wp": wp, "wkq": wkq, "mb": mb}
    if bp is not None:
        m["bp"] = np.asarray(bp).astype(bf16).reshape(1, E)
    return m


def kernel(
    x, mask, query, Wp, bp, Wq, bq, Wk, bk, Wv, bv, Wo, bo, W2, b2, gamma, beta,
    _trace=False,
):
    x = np.asarray(x)
    mask = np.asarray(mask)
    BL = B // NCORES

    # Host-side folds (all tiny)
    qh = (np.asarray(query, np.float64) @ np.asarray(Wq, np.float64)
          + np.asarray(bq, np.float64)).reshape(H, D)
    wkq_scaled = np.einsum(
        "ehd,hd->eh",
        np.asarray(Wk, np.float64).reshape(E, H, D),
        qh,
    ) / np.sqrt(D)

    has_bp = bool(np.any(np.asarray(bp)))
    nc = build_nc(has_bp=has_bp)

    in_maps = []
    for c in range(NCORES):
        sl = slice(c * BL, (c + 1) * BL)
        in_maps.append(
            prepare_core_inputs(
                x[sl], mask[sl], np.asarray(Wp), wkq_scaled.astype(np.float32),
                bp=np.asarray(bp) if has_bp else None,
            )
        )

    res = run_bass_kernel_spmd(
        nc, in_maps, core_ids=list(range(NCORES)), trace=_trace
    )
    U = np.concatenate([r["u_out"] for r in res.results], axis=0)  # (B, H, E)
    Z = np.concatenate([r["z_out"] for r in res.results], axis=0)[..., :1]  # (B, H, 1)

    # Host epilogue in float64
    pooledH = U.astype(np.float64) / Z.astype(np.float64)  # (B, H, E)
    Wv64 = np.asarray(Wv, np.float64).reshape(E, H, D)
    pooled = np.einsum("bhe,ehd->bhd", pooledH, Wv64).reshape(B, E)
    pooled += np.asarray(bv, np.float64)
    pooled = pooled @ np.asarray(Wo, np.float64) + np.asarray(bo, np.float64)
    out = pooled @ np.asarray(W2, np.float64) + np.asarray(b2, np.float64)
    mu = out.mean(-1, keepdims=True)
    var = out.var(-1, keepdims=True)
    out = (out - mu) / np.sqrt(var + 1e-5) * np.asarray(gamma, np.float64) + np.asarray(
        beta, np.float64
    )
    out_f32 = out.astype(np.float32)
    if _trace:
        return out_f32, res
    return out_f32



# revision 10
# speedup vs baseline: 1.1569x; 1.1569x over previous
"""Trainium2 Bass kernel for nn_AttentionPoolingTemporalEncoder.

Strategy (data-parallel over batch, 8 cores, 4 batch rows each):
  device:  h = relu(x @ Wp)               (bf16 matmuls, x pre-transposed on host)
           scores = h @ ((Wk @ qh)/sqrt(D))   (bk shifts cancel in softmax)
           p = exp(scores + maskbias)     (no running max; scores are O(5))
           U[h,:] = sum_s p[s,h] * h[s,:] ; Z[h] = sum_s p[s,h]
  host:    pooled = (U/Z) @ Wv (+bv) per head; @Wo+bo; @W2+b2; LayerNorm.
"""

import sys
import threading

import numpy as np

sys.path.insert(0, "/opt/trn_rl_repo")

from contextlib import ExitStack

import concourse.tile as tile
from concourse import bacc, mybir
from concourse.bass_utils import run_bass_kernel_spmd
from concourse.masks import make_identity


def _ensure_axon_ntff_hook_module():
    """Some images lack ``antenv.axon_hooks``; concourse imports it
    unconditionally when tracing is requested (e.g. via BASS_TRACE).
    Provide a minimal stand-in so that path degrades to no-trace
    instead of crashing."""
    try:
        from antenv import axon_hooks  # noqa: F401

        return
    except ImportError:
        pass
    import types

    mod = types.ModuleType("antenv.axon_hooks")
    mod._hook = None

    def set_axon_ntff_profile_hook(h):
        mod._hook = h

    def get_axon_ntff_profile_hook():
        return mod._hook

    mod.set_axon_ntff_profile_hook = set_axon_ntff_profile_hook
    mod.get_axon_ntff_profile_hook = get_axon_ntff_profile_hook
    sys.modules["antenv.axon_hooks"] = mod
    try:
        import antenv

        antenv.axon_hooks = mod
    except ImportError:
        pass


_ensure_axon_ntff_hook_module()

# Problem sizes (hardcoded per spec)
B, S, IN_DIM, E, H = 32, 4096, 1024, 512, 8
D = E // H
NCORES = 8
P = 128

_nc_cache = {}
_nc_lock = threading.Lock()


def build_nc(BL=B // NCORES, S_=S, I_=IN_DIM, has_bp=False, no_mask=False, trace_label=""):
    """Build + compile the per-core Bass program.

    BL: batch rows per core. S_: sequence length. I_: input dim.
    has_bp: emit the extra K=1 matmul adding the input-projection bias.
    """
    key = (BL, S_, I_, has_bp, no_mask)
    with _nc_lock:
        if key in _nc_cache:
            return _nc_cache[key]

    IC = I_ // P        # input-dim chunks
    EC = E // P         # embed-dim chunks
    S_TILES = S_ // P   # sequence tiles per batch row
    S_BLK = min(1024, S_)
    BLKS = S_ // S_BLK
    TPB = S_BLK // P    # s-tiles per DMA block

    f32 = mybir.dt.float32
    bf16 = mybir.dt.bfloat16
    fp8 = mybir.dt.float8e4
    DR = mybir.MatmulPerfMode.DoubleRow
    RELU = mybir.ActivationFunctionType.Relu
    EXP = mybir.ActivationFunctionType.Exp
    COPY = mybir.ActivationFunctionType.Copy

    nc = bacc.Bacc(
        "TRN2",
        target_bir_lowering=False,
        debug=False,
        enable_asserts=False,
        num_devices=NCORES,
    )

    # DRAM I/O (per-core shapes). Matmul operands are bf16 (host-cast):
    # halves HBM traffic and runs the PE at full rate.
    xt = nc.dram_tensor("xt", [BL, IC, P, S_], fp8, kind="ExternalInput").ap()
    wp = nc.dram_tensor("wp", [IC, P, E], fp8, kind="ExternalInput").ap()
    wkq = nc.dram_tensor("wkq", [EC, P, H], bf16, kind="ExternalInput").ap()
    mb = nc.dram_tensor("mb", [BL, P, S_TILES], f32, kind="ExternalInput").ap()
    if has_bp:
        bp_d = nc.dram_tensor("bp", [1, E], bf16, kind="ExternalInput").ap()
    u_out = nc.dram_tensor("u_out", [BL, H, E], f32, kind="ExternalOutput").ap()
    z_out = nc.dram_tensor("z_out", [BL, H, 1], f32, kind="ExternalOutput").ap()

    with tile.TileContext(nc) as tc, ExitStack() as ctx:
        const = ctx.enter_context(tc.tile_pool(name="const", bufs=1))
        xp = ctx.enter_context(tc.tile_pool(name="xp", bufs=3))
        hp = ctx.enter_context(tc.tile_pool(name="hp", bufs=4))
        htp = ctx.enter_context(tc.tile_pool(name="htp", bufs=3))
        pp = ctx.enter_context(tc.tile_pool(name="pp", bufs=3))
        mbp = ctx.enter_context(tc.tile_pool(name="mbp", bufs=2))
        uzp = ctx.enter_context(tc.tile_pool(name="uzp", bufs=2))
        ps_h = ctx.enter_context(tc.tile_pool(name="ps_h", bufs=4, space="PSUM"))
        ps_s = ctx.enter_context(tc.tile_pool(name="ps_s", bufs=1, space="PSUM"))
        ps_u = ctx.enter_context(tc.tile_pool(name="ps_u", bufs=2, space="PSUM"))
        ps_z = ctx.enter_context(tc.tile_pool(name="ps_z", bufs=1, space="PSUM"))

        # Resident constants
        wp_sb = const.tile([P, IC, E], fp8)
        nc.sync.dma_start(wp_sb[:], wp.rearrange("c p e -> p c e"))
        wkq_sb = const.tile([P, EC, H], bf16)
        nc.sync.dma_start(wkq_sb[:], wkq.rearrange("c p h -> p c h"))
        ones_t = const.tile([P, 2], bf16)
        nc.gpsimd.memset(ones_t[:], 1.0)
        if has_bp:
            ones_row = const.tile([1, P], bf16)
            nc.gpsimd.memset(ones_row[:], 1.0)
            bp_sb = const.tile([1, E], bf16)
            nc.sync.dma_start(bp_sb[:], bp_d[:])

        # Chunked x prefetch (1024 s = 8 tiles per chunk), issued ahead so
        # loads never queue behind the per-tile transposes.
        SC = min(1024, S_)
        NCH = S_ // SC
        chunks = [(bb, cc) for bb in range(BL) for cc in range(NCH)]

        def load_chunk(idx):
            bb, cc = chunks[idx]
            xt_c = xp.tile([P, IC, SC], fp8, tag="xchunk")
            nc.sync.dma_start(
                xt_c[:],
                xt[bb, :, :, cc * SC : (cc + 1) * SC].rearrange("c p s -> p c s"),
            )
            return xt_c

        # distance-2 prefetch: two chunks in flight ahead of the consumer
        bufq = [load_chunk(0)]
        if len(chunks) > 1:
            bufq.append(load_chunk(1))
        chunk_idx = 1

        for b in range(BL):
            mb_t = mbp.tile([P, S_TILES], f32)
            nc.gpsimd.dma_start(mb_t[:], mb[b])
            u_ps = ps_u.tile([H, E], f32)
            z_ps = ps_z.tile([H, 2], f32)

            # Software-pipelined tails (depth 2): tile t's attention tail
            # (scores/exp/U/Z) is emitted 1-2 tiles later; all pending tails
            # flush right before a chunk boundary so the PE has work to chew
            # while a late x-chunk DMA completes.
            pending = []  # [(t, h_se, ht_sb), ...]

            def emit_tail(pend):
                t_, h_se_, ht_sb_ = pend
                # scores[s,h] = sum_e h[s,e] wkq[e,h]
                sc_ps = ps_s.tile([P, H], f32)
                for ec in range(EC):
                    nc.tensor.matmul(
                        sc_ps[:],
                        ht_sb_[:, ec, :],
                        wkq_sb[:, ec, :],
                        start=(ec == 0),
                        stop=(ec == EC - 1),
                    )
                # p = exp(scores + maskbias); maskbias = 0 for unmasked, -1e4
                # for masked positions (additive bias port, per-partition).
                p_sb = pp.tile([P, H], bf16)
                nc.scalar.activation(
                    p_sb[:], sc_ps[:], EXP, bias=mb_t[:, t_ : t_ + 1]
                )
                nc.tensor.matmul(
                    u_ps[:],
                    p_sb[:],
                    h_se_[:],
                    start=(t_ == 0),
                    stop=(t_ == S_TILES - 1),
                    skip_group_check=True,
                )
                nc.tensor.matmul(
                    z_ps[:],
                    p_sb[:],
                    ones_t[:],
                    start=(t_ == 0),
                    stop=(t_ == S_TILES - 1),
                    skip_group_check=True,
                )

            for t in range(S_TILES):
                    TPC = SC // P
                    if t % TPC == 0:
                        # consume the next chunk; keep two loads in flight
                        x_sb = bufq.pop(0)
                        if chunk_idx + 1 < len(chunks):
                            chunk_idx += 1
                            bufq.append(load_chunk(chunk_idx))
                    # h = relu(x @ Wp): accumulate i-chunk PAIRS into PSUM via
                    # fp8 DoubleRow (2 MACs/cell/cycle, K=256 per matmul)
                    h_ps = ps_h.tile([P, E], f32)
                    for cp in range(IC // 2):
                        nc.tensor.matmul(
                            h_ps[:],
                            x_sb[:, 2 * cp : 2 * cp + 2, (t % TPC) * P : (t % TPC + 1) * P],
                            wp_sb[:, 2 * cp : 2 * cp + 2, :],
                            start=(cp == 0),
                            stop=(cp == IC // 2 - 1) and not has_bp,
                            perf_mode=DR,
                        )
                    if has_bp:
                        nc.tensor.matmul(
                            h_ps[:],
                            ones_row[:],
                            bp_sb[:],
                            start=False,
                            stop=True,
                        )
                    h_se = hp.tile([P, E], bf16)
                    nc.scalar.activation(h_se[:], h_ps[:], RELU)

                    # hT via one batched DMA XBAR transpose, SBUF -> SBUF:
                    # ht_sb[e_in, ec, s] = h_se[s, ec*128 + e_in]
                    ht_sb = htp.tile([P, EC, P], bf16)
                    nc.sync.dma_start_transpose(ht_sb[:], h_se[:])

                    pending.append((t, h_se, ht_sb))
                    if len(pending) > 1:
                        emit_tail(pending.pop(0))
            while pending:
                emit_tail(pending.pop(0))

            u_sb = uzp.tile([H, E], f32, tag="u_sb")
            z_sb = uzp.tile([H, 1], f32, tag="z_sb")
            nc.vector.tensor_copy(u_sb[:], u_ps[:])
            nc.vector.tensor_copy(z_sb[:], z_ps[:, 0:1])
            nc.sync.dma_start(u_out[b], u_sb[:])
            nc.sync.dma_start(z_out[b], z_sb[:])

    nc.compile()
    with _nc_lock:
        _nc_cache[key] = nc
    return nc


def prepare_core_inputs(x, mask, Wp, wkq_scaled, bp=None):
    """Host-side packing for ONE core's shard.

    x: (BL, S, IN_DIM) fp32; mask: (BL, S) int; wkq_scaled: (E, H) fp32.
    """
    import ml_dtypes

    bf16 = ml_dtypes.bfloat16
    fp8 = ml_dtypes.float8_e4m3
    BL_, S_, I_ = x.shape
    IC = I_ // P
    EC = E // P
    # xt[b, c, i_in, s] = x[b, s, c*128+i_in].  fp8 e4m3 (TRN variant, max
    # 240): |x| <~ 6 so no clipping needed.
    xt = np.ascontiguousarray(
        x.reshape(BL_, S_, IC, P).transpose(0, 2, 3, 1)
    ).astype(fp8)
    # Wp prescaled x32 so fp8 weights sit in the normal range (std ~1);
    # h on device is 32*h_true, compensated on the host (U /= 32) and in
    # wkq (wkq_dev = wkq_true/32).
    wp = np.ascontiguousarray(Wp.reshape(IC, P, E) * np.float32(32.0)).astype(fp8)
    wkq = np.ascontiguousarray(wkq_scaled.reshape(EC, P, H)).astype(bf16)
    # additive mask bias packed [BL, P, S_TILES]: 0 where kept, -1e4 where
    # masked (exp(-1e4 + s) underflows to exactly 0)
    mb = np.ascontiguousarray(
        ((mask.astype(np.float32) - 1.0) * 1.0e4)
        .reshape(BL_, S_ // P, P)
        .transpose(0, 2, 1)
    ).astype(np.float32)
    m = {"xt": xt, "wp": wp, "wkq": wkq, "mb": mb}
    if bp is not None:
        # device h is 32*h_true, so the pre-relu bias must be 32*bp
        m["bp"] = (np.asarray(bp) * np.float32(32.0)).astype(bf16).reshape(1, E)
    return m


def kernel(
    x, mask, query, Wp, bp, Wq, bq, Wk, bk, Wv, bv, Wo, bo, W2, b2, gamma, beta,
    _trace=False,
):
    x = np.asarray(x)
    mask = np.asarray(mask)
    BL = B // NCORES

    # Host-side folds (all tiny)
    qh = (np.asarray(query, np.float64) @ np.asarray(Wq, np.float64)
          + np.asarray(bq, np.float64)).reshape(H, D)
    wkq_scaled = np.einsum(
        "ehd,hd->eh",
        np.asarray(Wk, np.float64).reshape(E, H, D),
        qh,
    ) / np.sqrt(D)

    has_bp = bool(np.any(np.asarray(bp)))
    nc = build_nc(has_bp=has_bp)

    in_maps = []
    for c in range(NCORES):
        sl = slice(c * BL, (c + 1) * BL)
        in_maps.append(
            prepare_core_inputs(
                x[sl], mask[sl], np.asarray(Wp),
                (wkq_scaled / 32.0).astype(np.float32),
                bp=np.asarray(bp) if has_bp else None,
            )
        )

    res = run_bass_kernel_spmd(
        nc, in_maps, core_ids=list(range(NCORES)), trace=_trace
    )
    U = np.concatenate([r["u_out"] for r in res.results], axis=0)  # (B, H, E)
    Z = np.concatenate([r["z_out"] for r in res.results], axis=0)[..., :1]  # (B, H, 1)

    # Host epilogue in float64 (device h was 32*h_true -> U is 32*U_true)
    pooledH = U.astype(np.float64) / (32.0 * Z.astype(np.float64))  # (B, H, E)
    Wv64 = np.asarray(Wv, np.float64).reshape(E, H, D)
    pooled = np.einsum("bhe,ehd->bhd", pooledH, Wv64).reshape(B, E)
    pooled += np.asarray(bv, np.float64)
    pooled = pooled @ np.asarray(Wo, np.float64) + np.asarray(bo, np.float64)
    out = pooled @ np.asarray(W2, np.float64) + np.asarray(b2, np.float64)
    mu = out.mean(-1, keepdims=True)
    var = out.var(-1, keepdims=True)
    out = (out - mu) / np.sqrt(var + 1e-5) * np.asarray(gamma, np.float64) + np.asarray(
        beta, np.float64
    )
    out_f32 = out.astype(np.float32)
    if _trace:
        return out_f32, res
    return out_f32

